# revision 31
# baseline (speedup 1.0000x reference)
"""ClassAlignmentLoss kernel for 8 TRN2 NeuronCores (Bass/Tile), v2b.

Data-parallel over N per domain.  Host pre-casts the feature shard to fp8e4
in the exact SBUF layout ([128, dom*tile*257], col 256 of each tile row is
a 1.0 used by the fused cross-term), plus fp8 one-hot matrices (oh for the
phase-1 segment-sum stationary, ohT for the phase-2 center-select
stationary).  Device DMA is a flat contiguous copy.

Phase 1: per-tile one-hot matmuls accumulate per-class sums in PSUM while
the DMA streams; ScalarE+VectorE compute ||f||^2 for the B-half tiles.
Domain 0's sums all-reduce first (fp32); domains 1-2 follow in a second
all-reduce that overlaps domain 0's phase 2.  Phase 2 computes
q_i = ||f_i - c_{l_i}||^2 two ways to balance engines: A tiles build
(c_sel - f) in PSUM (negI + ohT matmuls) and square-accumulate per tile on
ScalarE; B tiles do a fused DVE scalar_tensor_tensor of F_aug against the
PSUM (-2c | ||c||^2) selection.  q ships to the host, which does
sqrt/mean and the tiny center-distance terms in float64.
"""

import numpy as np

N_DOM = 3
N = 65536
D = 256
C = 64
N_CORES = 8
NSH = N // N_CORES
P = 128
TILES = NSH // P
TT = N_DOM * TILES
DS = D + 1
WIN = 16
ASUB = 4

ALPHA, BETA, GAMA = 1.0, 1.0, 1.0


def build(warmup_cc=False):
    import concourse.bass as bass
    import concourse.bacc as bacc
    import concourse.mybir as mybir
    import concourse.tile as tile

    dt = mybir.dt
    Alu = mybir.AluOpType
    Act = mybir.ActivationFunctionType

    nc = bacc.Bacc(
        "TRN2",
        target_bir_lowering=False,
        debug=False,
        num_devices=N_CORES,
    )

    feat = nc.dram_tensor("feat", [P, TT * DS], dt.float8e4, kind="ExternalInput")
    oh_in = nc.dram_tensor("oh", [P, TT * C], dt.float8e4, kind="ExternalInput")
    ohT_in = nc.dram_tensor("ohT", [C, N_DOM * NSH], dt.float8e4, kind="ExternalInput")
    negI_in = nc.dram_tensor("negI", [P, P], dt.float8e4, kind="ExternalInput")
    invc_in = nc.dram_tensor("invc", [C, 8], dt.float32, kind="ExternalInput")
    out_sums = nc.dram_tensor("out_sums", [C, N_DOM * D], dt.float32, kind="ExternalOutput")
    out_q = nc.dram_tensor("out_q", [P, TT], dt.float32, kind="ExternalOutput")

    rg = [list(range(N_CORES))]

    with tile.TileContext(nc) as tc:
        with (
            tc.tile_pool(name="persist", bufs=1) as pp,
            tc.tile_pool(name="cent", bufs=2) as centp,
            tc.tile_pool(name="sq", bufs=3) as sqp,
            tc.tile_pool(name="trA", bufs=3) as trap,
            tc.tile_pool(name="trB", bufs=3) as trbp,
            tc.tile_pool(name="pseg", bufs=1, space="PSUM") as psegp,
            tc.tile_pool(name="pa", bufs=2, space="PSUM") as pap,
            tc.tile_pool(name="pbc", bufs=2, space="PSUM") as pbcp,
            tc.tile_pool(name="dram", bufs=1, space="DRAM") as dramp,
        ):
            f8 = pp.tile([P, TT * DS], dt.float8e4, tag="f8")
            oh = pp.tile([P, TT * C], dt.float8e4, tag="oh")
            ohT = pp.tile([C, N_DOM * NSH], dt.float8e4, tag="ohT")
            negI = pp.tile([P, P], dt.float8e4, tag="negI")
            invc = pp.tile([C, 8], dt.float32, tag="invc")
            qf = pp.tile([P, TT], dt.float32, tag="qf")
            qx = pp.tile([P, TT], dt.float32, tag="qx")
            qq = pp.tile([P, TT], dt.float32, tag="qq")
            s_all = pp.tile([C, N_DOM * D], dt.float32, tag="s_all")
            s_glob = pp.tile([C, N_DOM * D], dt.float32, tag="s_glob")

            nc.vector.memset(qf[:], 0.0)

            nc.sync.dma_start(negI[:], negI_in[:])
            nc.sync.dma_start(invc[:], invc_in[:])
            for d in range(N_DOM):
                ob = d * TILES * C
                nc.sync.dma_start(oh[:, ob:ob + TILES * C], oh_in[:, ob:ob + TILES * C])
                fb = d * TILES * DS
                nch = 4
                step = TILES * DS // nch
                for k in range(nch):
                    a = fb + k * step
                    nc.sync.dma_start(f8[:, a:a + step], feat[:, a:a + step])
                tb = d * NSH
                nc.sync.dma_start(ohT[:, tb:tb + NSH], ohT_in[:, tb:tb + NSH])

            cc_in = [None, None]
            cc_out = [None, None]
            for d in range(N_DOM):
                pseg = psegp.tile([C, D], dt.float32, tag="pseg")
                for t in range(TILES):
                    g = d * TILES + t
                    nc.tensor.matmul(
                        pseg[:],
                        oh[:, g * C:(g + 1) * C],
                        f8[:, g * DS:g * DS + D],
                        start=(t == 0),
                        stop=(t == TILES - 1),
                    )
                for w in range(TILES // WIN):
                    g0 = d * TILES + w * WIN + 8
                    fa = f8[:, g0 * DS:(g0 + 8) * DS].rearrange(
                        "p (t m) -> p t m", m=DS
                    )[:, :, 0:D]
                    sq = sqp.tile([P, 8 * D], dt.float16, tag="sqf")
                    sq3 = sq[:].rearrange("p (t m) -> p t m", m=D)
                    nc.scalar.activation(sq3, fa, Act.Square)
                    nc.vector.reduce_sum(
                        qf[:, g0:g0 + 8], sq3, axis=mybir.AxisListType.X
                    )
                nc.scalar.copy(s_all[:, d * D:(d + 1) * D], pseg[:])
                if d == 0:
                    cc_in[0] = dramp.tile([C, D], dt.float32, tag="cc_in0", name="cc_in0")
                    cc_out[0] = dramp.tile([C, D], dt.float32, tag="cc_out0", name="cc_out0")
                    nc.sync.dma_start(cc_in[0][:], s_all[:, 0:D])
                    nc.gpsimd.collective_compute(
                        "AllReduce", Alu.add, replica_groups=rg,
                        ins=[cc_in[0].opt()], outs=[cc_out[0].opt()],
                    )
                    nc.sync.dma_start(s_glob[:, 0:D], cc_out[0][:])
                    nc.sync.dma_start(out_sums[:, 0:D], cc_out[0][:])
                elif d == 2:
                    cc_in[1] = dramp.tile([C, 2 * D], dt.float32, tag="cc_in1", name="cc_in1")
                    cc_out[1] = dramp.tile([C, 2 * D], dt.float32, tag="cc_out1", name="cc_out1")
                    nc.sync.dma_start(cc_in[1][:], s_all[:, D:3 * D])
                    nc.gpsimd.collective_compute(
                        "AllReduce", Alu.add, replica_groups=rg,
                        ins=[cc_in[1].opt()], outs=[cc_out[1].opt()],
                    )
                    nc.sync.dma_start(s_glob[:, D:3 * D], cc_out[1][:])
                    nc.sync.dma_start(out_sums[:, D:3 * D], cc_out[1][:])

            # PE bridge: keep the array busy across the all-reduce wait so
            # phase 2 starts at 2.4GHz; results go to an unused PSUM bank.
            pwarm = psegp.tile([C, D], dt.float32, tag="pwarm")
            for r in range(240):
                g = (r * 11) % TT
                nc.tensor.matmul(
                    pwarm[:],
                    oh[:, g * C:(g + 1) * C],
                    f8[:, g * DS:g * DS + D],
                    start=(r == 0),
                    stop=False,
                )

            nwarm = [240]

            def warm_tick(n=1):
                for _ in range(n):
                    g = (nwarm[0] * 11) % TT
                    nwarm[0] += 1
                    nc.tensor.matmul(
                        pwarm[:],
                        oh[:, g * C:(g + 1) * C],
                        f8[:, g * DS:g * DS + D],
                        start=False, stop=False,
                    )

            for d in range(N_DOM):
                cpos = centp.tile([C, D], dt.float16, tag="cpos")
                nc.vector.tensor_scalar(
                    cpos[:], s_glob[:, d * D:(d + 1) * D],
                    invc[:, 2 * d:2 * d + 1], None, Alu.mult,
                )
                caug = centp.tile([C, DS], dt.float16, tag="caug")
                nc.vector.tensor_scalar(
                    caug[:, 0:D], s_glob[:, d * D:(d + 1) * D],
                    invc[:, 2 * d + 1:2 * d + 2], None, Alu.mult,
                )
                ctr = centp.tile([C, D], dt.float16, tag="ctrash")
                cnorm = centp.tile([C, 1], dt.float32, tag="cnorm")
                nc.vector.scalar_tensor_tensor(
                    ctr[:], cpos[:], 1.0, cpos[:],
                    Alu.mult, Alu.mult, accum_out=cnorm[:],
                )
                nc.vector.tensor_copy(caug[:, D:DS], cnorm[:])

                for w in range(TILES // WIN):
                    w0 = d * TILES + w * WIN
                    for s in range(8 // ASUB):
                        pa = pap.tile([P, ASUB * D], dt.float32, tag="pa")
                        for j in range(ASUB):
                            g = w0 + s * ASUB + j
                            tloc = w * WIN + s * ASUB + j
                            nc.tensor.matmul(
                                pa[:, j * D:(j + 1) * D],
                                negI[:],
                                f8[:, g * DS:g * DS + D],
                                start=True, stop=False,
                            )
                            nc.tensor.matmul(
                                pa[:, j * D:(j + 1) * D],
                                ohT[:, d * NSH + tloc * P:
                                     d * NSH + (tloc + 1) * P],
                                cpos[:],
                                start=False, stop=True,
                            )
                        for j in range(ASUB):
                            g = w0 + s * ASUB + j
                            ta = trap.tile([P, D], dt.float16, tag="ta")
                            nc.scalar.activation(
                                ta[:], pa[:, j * D:(j + 1) * D], Act.Square,
                                accum_out=qx[:, g:g + 1],
                            )
                    warm_tick(2)
                    for j in range(8):
                        g = w0 + 8 + j
                        tloc = w * WIN + 8 + j
                        pbc = pbcp.tile([P, DS], dt.float32, tag="pbc")
                        nc.tensor.matmul(
                            pbc[:],
                            ohT[:, d * NSH + tloc * P: d * NSH + (tloc + 1) * P],
                            caug[:],
                            start=True, stop=True,
                        )
                        tb = trbp.tile([P, DS], dt.float16, tag="tb")
                        nc.vector.scalar_tensor_tensor(
                            tb[:], f8[:, g * DS:(g + 1) * DS], 1.0, pbc[:],
                            Alu.mult, Alu.mult,
                            accum_out=qx[:, g:g + 1],
                        )

            warm_tick(1)
            nc.tensor.matmul(pwarm[:], oh[:, 0:C], f8[:, 0:D],
                             start=False, stop=True)
            nc.vector.tensor_tensor(qq[:], qf[:], qx[:], Alu.add)
            nc.sync.dma_start(out_q[:, :], qq[:])

    nc.compile()
    return nc


_CACHED = {}


def _get_nc(key=(False,)):
    if key not in _CACHED:
        _CACHED[key] = build(*key)
    return _CACHED[key]


def shard_inputs(features, labels):
    """Host prep: fp8 SBUF-layout features + one-hots, inverse counts."""
    import ml_dtypes

    f8dt = ml_dtypes.float8_e4m3
    features = np.asarray(features, dtype=np.float32)
    labels = np.asarray(labels, dtype=np.int32)

    counts = np.stack(
        [np.bincount(labels[d], minlength=C) for d in range(N_DOM)]
    ).astype(np.float64)
    cnt = np.maximum(counts, 1.0)
    invc_full = np.zeros((C, 8), dtype=np.float32)
    for d in range(N_DOM):
        invc_full[:, 2 * d] = (1.0 / cnt[d]).astype(np.float32)
        invc_full[:, 2 * d + 1] = (-2.0 / cnt[d]).astype(np.float32)

    negI = (-np.eye(P)).astype(f8dt)

    in_maps = []
    for c in range(N_CORES):
        fl = features[:, c * NSH:(c + 1) * NSH, :]
        lb = labels[:, c * NSH:(c + 1) * NSH]
        fc = np.ones((P, N_DOM, TILES, DS), dtype=f8dt)
        fc[:, :, :, 0:D] = (
            fl.reshape(N_DOM, TILES, P, D).transpose(2, 0, 1, 3)
        ).astype(f8dt)
        lr = lb.reshape(N_DOM, TILES, P)
        oh_u8 = (lr[:, :, :, None] == np.arange(C)[None, None, None, :])
        oh_u8 = (oh_u8.astype(np.uint8) * 0x38).transpose(2, 0, 1, 3)
        ohT_u8 = (lb[:, None, :] == np.arange(C)[None, :, None])
        ohT_u8 = (ohT_u8.astype(np.uint8) * 0x38).transpose(1, 0, 2)
        in_maps.append({
            "feat": np.ascontiguousarray(fc.reshape(P, TT * DS)),
            "oh": np.ascontiguousarray(oh_u8.reshape(P, TT * C)).view(f8dt),
            "ohT": np.ascontiguousarray(ohT_u8.reshape(C, N_DOM * NSH)).view(f8dt),
            "negI": negI,
            "invc": invc_full,
        })
    return in_maps, counts


def finish_host(out_maps, counts, labels=None):
    """Combine per-core outputs into the scalar loss (numpy, float64)."""
    comp = np.zeros(N_DOM, dtype=np.float64)
    for m in out_maps:
        q = m["out_q"].astype(np.float64).reshape(P, N_DOM, TILES)
        dist = np.sqrt(np.maximum(q, 0.0))
        comp += dist.sum(axis=(0, 2))
    comp /= N

    S = out_maps[0]["out_sums"].astype(np.float64)
    cnt = np.maximum(counts, 1.0)
    sep = np.zeros(N_DOM, dtype=np.float64)
    centers = np.zeros((N_DOM, C, D), dtype=np.float64)
    for d in range(N_DOM):
        centers[d] = S[:, d * D:(d + 1) * D] / cnt[d][:, None]
        cd = centers[d]
        sq = ((cd[:, None, :] - cd[None, :, :]) ** 2).sum(-1)
        dist = np.sqrt(np.maximum(sq, 0.0))
        np.fill_diagonal(dist, 0.0)
        sep[d] = dist.sum() / (C * (C - 1))

    intra = (BETA * comp.sum() - ALPHA * sep.sum()) / N_DOM
    inter = 0.0
    n_pairs = 0
    for i in range(N_DOM):
        for j in range(i + 1, N_DOM):
            inter += np.sqrt(((centers[i] - centers[j]) ** 2).sum()) / C
            n_pairs += 1
    inter /= n_pairs
    return np.float32(GAMA * intra + inter)


def kernel(features, labels):
    from concourse.bass_utils import run_bass_kernel_spmd

    nc = _get_nc()
    in_maps, counts = shard_inputs(features, labels)
    res = run_bass_kernel_spmd(nc, in_maps, core_ids=list(range(N_CORES)))
    return finish_host(res.results, counts)


# revision 32
# speedup vs baseline: 1.0891x; 1.0891x over previous
"""ClassAlignmentLoss kernel for 8 TRN2 NeuronCores (Bass/Tile), v2b.

Data-parallel over N per domain.  Host pre-casts the feature shard to fp8e4
in the exact SBUF layout ([128, dom*tile*257], col 256 of each tile row is
a 1.0 used by the fused cross-term), plus fp8 one-hot matrices (oh for the
phase-1 segment-sum stationary, ohT for the phase-2 center-select
stationary).  Device DMA is a flat contiguous copy.

Phase 1: per-tile one-hot matmuls accumulate per-class sums in PSUM while
the DMA streams; ScalarE+VectorE compute ||f||^2 for the B-half tiles.
Domain 0's sums all-reduce first (fp32); domains 1-2 follow in a second
all-reduce that overlaps domain 0's phase 2.  Phase 2 computes
q_i = ||f_i - c_{l_i}||^2 two ways to balance engines: A tiles build
(c_sel - f) in PSUM (negI + ohT matmuls) and square-accumulate per tile on
ScalarE; B tiles do a fused DVE scalar_tensor_tensor of F_aug against the
PSUM (-2c | ||c||^2) selection.  q ships to the host, which does
sqrt/mean and the tiny center-distance terms in float64.
"""

import numpy as np

N_DOM = 3
N = 65536
D = 256
C = 64
N_CORES = 8
NSH = N // N_CORES
P = 128
TILES = NSH // P
TT = N_DOM * TILES
DS = D + 1
WIN = 16
ASUB = 4

ALPHA, BETA, GAMA = 1.0, 1.0, 1.0


def build(warmup_cc=False):
    import concourse.bass as bass
    import concourse.bacc as bacc
    import concourse.mybir as mybir
    import concourse.tile as tile

    dt = mybir.dt
    Alu = mybir.AluOpType
    Act = mybir.ActivationFunctionType

    nc = bacc.Bacc(
        "TRN2",
        target_bir_lowering=False,
        debug=False,
        num_devices=N_CORES,
    )

    feat = nc.dram_tensor("feat", [P, TT * DS], dt.float8e4, kind="ExternalInput")
    oh_in = nc.dram_tensor("oh", [P, TT * C], dt.float8e4, kind="ExternalInput")
    ohT_in = nc.dram_tensor("ohT", [C, N_DOM * NSH], dt.float8e4, kind="ExternalInput")
    negI_in = nc.dram_tensor("negI", [P, P], dt.float8e4, kind="ExternalInput")
    invc_in = nc.dram_tensor("invc", [C, 8], dt.float32, kind="ExternalInput")
    out_sums = nc.dram_tensor("out_sums", [C, N_DOM * D], dt.float32, kind="ExternalOutput")
    out_q = nc.dram_tensor("out_q", [P, TT], dt.float32, kind="ExternalOutput")

    rg = [list(range(N_CORES))]

    with tile.TileContext(nc) as tc:
        with (
            tc.tile_pool(name="persist", bufs=1) as pp,
            tc.tile_pool(name="cent", bufs=2) as centp,
            tc.tile_pool(name="sq", bufs=3) as sqp,
            tc.tile_pool(name="trA", bufs=3) as trap,
            tc.tile_pool(name="trB", bufs=3) as trbp,
            tc.tile_pool(name="pseg", bufs=1, space="PSUM") as psegp,
            tc.tile_pool(name="pa", bufs=2, space="PSUM") as pap,
            tc.tile_pool(name="pbc", bufs=2, space="PSUM") as pbcp,
            tc.tile_pool(name="dram", bufs=1, space="DRAM") as dramp,
        ):
            f8 = pp.tile([P, TT * DS], dt.float8e4, tag="f8")
            oh = pp.tile([P, TT * C], dt.float8e4, tag="oh")
            ohT = pp.tile([C, N_DOM * NSH], dt.float8e4, tag="ohT")
            negI = pp.tile([P, P], dt.float8e4, tag="negI")
            invc = pp.tile([C, 8], dt.float32, tag="invc")
            qf = pp.tile([P, TT], dt.float32, tag="qf")
            qx = pp.tile([P, TT], dt.float32, tag="qx")
            qq = pp.tile([P, TT], dt.float32, tag="qq")
            s_all = pp.tile([C, N_DOM * D], dt.float32, tag="s_all")
            s_glob = pp.tile([C, N_DOM * D], dt.float32, tag="s_glob")

            nc.vector.memset(qf[:], 0.0)

            nc.sync.dma_start(negI[:], negI_in[:])
            nc.sync.dma_start(invc[:], invc_in[:])
            for d in range(N_DOM):
                ob = d * TILES * C
                nc.sync.dma_start(oh[:, ob:ob + TILES * C], oh_in[:, ob:ob + TILES * C])
                fb = d * TILES * DS
                nch = 4
                step = TILES * DS // nch
                for k in range(nch):
                    a = fb + k * step
                    nc.sync.dma_start(f8[:, a:a + step], feat[:, a:a + step])
                tb = d * NSH
                nc.sync.dma_start(ohT[:, tb:tb + NSH], ohT_in[:, tb:tb + NSH])

            cc_in = [None, None]
            cc_out = [None, None]
            for d in range(N_DOM):
                pseg = psegp.tile([C, D], dt.float32, tag="pseg")
                for t in range(TILES):
                    g = d * TILES + t
                    nc.tensor.matmul(
                        pseg[:],
                        oh[:, g * C:(g + 1) * C],
                        f8[:, g * DS:g * DS + D],
                        start=(t == 0),
                        stop=(t == TILES - 1),
                    )
                for w in range(TILES // WIN):
                    g0 = d * TILES + w * WIN + 8
                    fa = f8[:, g0 * DS:(g0 + 8) * DS].rearrange(
                        "p (t m) -> p t m", m=DS
                    )[:, :, 0:D]
                    sq = sqp.tile([P, 8 * D], dt.float16, tag="sqf")
                    sq3 = sq[:].rearrange("p (t m) -> p t m", m=D)
                    nc.scalar.activation(sq3, fa, Act.Square)
                    nc.vector.reduce_sum(
                        qf[:, g0:g0 + 8], sq3, axis=mybir.AxisListType.X
                    )
                nc.scalar.copy(s_all[:, d * D:(d + 1) * D], pseg[:])
                if d == 0:
                    cc_in[0] = dramp.tile([C, D], dt.float32, tag="cc_in0", name="cc_in0")
                    cc_out[0] = dramp.tile([C, D], dt.float32, tag="cc_out0", name="cc_out0")
                    nc.sync.dma_start(cc_in[0][:], s_all[:, 0:D])
                    nc.gpsimd.collective_compute(
                        "AllReduce", Alu.add, replica_groups=rg,
                        ins=[cc_in[0].opt()], outs=[cc_out[0].opt()],
                    )
                    nc.sync.dma_start(s_glob[:, 0:D], cc_out[0][:])
                    nc.sync.dma_start(out_sums[:, 0:D], cc_out[0][:])
                elif d == 2:
                    cc_in[1] = dramp.tile([C, 2 * D], dt.float32, tag="cc_in1", name="cc_in1")
                    cc_out[1] = dramp.tile([C, 2 * D], dt.float32, tag="cc_out1", name="cc_out1")
                    nc.sync.dma_start(cc_in[1][:], s_all[:, D:3 * D])
                    nc.gpsimd.collective_compute(
                        "AllReduce", Alu.add, replica_groups=rg,
                        ins=[cc_in[1].opt()], outs=[cc_out[1].opt()],
                    )
                    nc.sync.dma_start(s_glob[:, D:3 * D], cc_out[1][:])
                    nc.sync.dma_start(out_sums[:, D:3 * D], cc_out[1][:])

            # PE bridge: keep the array busy across the all-reduce wait so
            # phase 2 starts at 2.4GHz; results go to an unused PSUM bank.
            pwarm = psegp.tile([C, D], dt.float32, tag="pwarm")
            for r in range(200):
                g = (r * 11) % TT
                nc.tensor.matmul(
                    pwarm[:],
                    oh[:, g * C:(g + 1) * C],
                    f8[:, g * DS:g * DS + D],
                    start=(r == 0),
                    stop=False,
                )

            nwarm = [200]

            def warm_tick(n=1):
                for _ in range(n):
                    g = (nwarm[0] * 11) % TT
                    nwarm[0] += 1
                    nc.tensor.matmul(
                        pwarm[:],
                        oh[:, g * C:(g + 1) * C],
                        f8[:, g * DS:g * DS + D],
                        start=False, stop=False,
                    )

            for d in range(N_DOM):
                cpos = centp.tile([C, D], dt.float16, tag="cpos")
                nc.vector.tensor_scalar(
                    cpos[:], s_glob[:, d * D:(d + 1) * D],
                    invc[:, 2 * d:2 * d + 1], None, Alu.mult,
                )
                caug = centp.tile([C, DS], dt.float16, tag="caug")
                nc.vector.tensor_scalar(
                    caug[:, 0:D], s_glob[:, d * D:(d + 1) * D],
                    invc[:, 2 * d + 1:2 * d + 2], None, Alu.mult,
                )
                ctr = centp.tile([C, D], dt.float16, tag="ctrash")
                cnorm = centp.tile([C, 1], dt.float32, tag="cnorm")
                nc.vector.scalar_tensor_tensor(
                    ctr[:], cpos[:], 1.0, cpos[:],
                    Alu.mult, Alu.mult, accum_out=cnorm[:],
                )
                nc.vector.tensor_copy(caug[:, D:DS], cnorm[:])

                for w in range(TILES // WIN):
                    w0 = d * TILES + w * WIN
                    for s in range(8 // ASUB):
                        pa = pap.tile([P, ASUB * D], dt.float32, tag="pa")
                        for j in range(ASUB):
                            g = w0 + s * ASUB + j
                            tloc = w * WIN + s * ASUB + j
                            nc.tensor.matmul(
                                pa[:, j * D:(j + 1) * D],
                                negI[:],
                                f8[:, g * DS:g * DS + D],
                                start=True, stop=False,
                            )
                            nc.tensor.matmul(
                                pa[:, j * D:(j + 1) * D],
                                ohT[:, d * NSH + tloc * P:
                                     d * NSH + (tloc + 1) * P],
                                cpos[:],
                                start=False, stop=True,
                            )
                        for j in range(ASUB):
                            g = w0 + s * ASUB + j
                            ta = trap.tile([P, D], dt.float16, tag="ta")
                            nc.scalar.activation(
                                ta[:], pa[:, j * D:(j + 1) * D], Act.Square,
                                accum_out=qx[:, g:g + 1],
                            )
                    warm_tick(2)
                    for j in range(8):
                        g = w0 + 8 + j
                        tloc = w * WIN + 8 + j
                        pbc = pbcp.tile([P, DS], dt.float32, tag="pbc")
                        nc.tensor.matmul(
                            pbc[:],
                            ohT[:, d * NSH + tloc * P: d * NSH + (tloc + 1) * P],
                            caug[:],
                            start=True, stop=True,
                        )
                        tb = trbp.tile([P, DS], dt.float16, tag="tb")
                        nc.vector.scalar_tensor_tensor(
                            tb[:], f8[:, g * DS:(g + 1) * DS], 1.0, pbc[:],
                            Alu.mult, Alu.mult,
                            accum_out=qx[:, g:g + 1],
                        )

            warm_tick(1)
            nc.tensor.matmul(pwarm[:], oh[:, 0:C], f8[:, 0:D],
                             start=False, stop=True)
            nc.vector.tensor_tensor(qq[:], qf[:], qx[:], Alu.add)
            nc.sync.dma_start(out_q[:, :], qq[:])

    nc.compile()
    return nc


_CACHED = {}


def _get_nc(key=(False,)):
    if key not in _CACHED:
        _CACHED[key] = build(*key)
    return _CACHED[key]


def shard_inputs(features, labels):
    """Host prep: fp8 SBUF-layout features + one-hots, inverse counts."""
    import ml_dtypes

    f8dt = ml_dtypes.float8_e4m3
    features = np.asarray(features, dtype=np.float32)
    labels = np.asarray(labels, dtype=np.int32)

    counts = np.stack(
        [np.bincount(labels[d], minlength=C) for d in range(N_DOM)]
    ).astype(np.float64)
    cnt = np.maximum(counts, 1.0)
    invc_full = np.zeros((C, 8), dtype=np.float32)
    for d in range(N_DOM):
        invc_full[:, 2 * d] = (1.0 / cnt[d]).astype(np.float32)
        invc_full[:, 2 * d + 1] = (-2.0 / cnt[d]).astype(np.float32)

    negI = (-np.eye(P)).astype(f8dt)

    in_maps = []
    for c in range(N_CORES):
        fl = features[:, c * NSH:(c + 1) * NSH, :]
        lb = labels[:, c * NSH:(c + 1) * NSH]
        fc = np.ones((P, N_DOM, TILES, DS), dtype=f8dt)
        fc[:, :, :, 0:D] = (
            fl.reshape(N_DOM, TILES, P, D).transpose(2, 0, 1, 3)
        ).astype(f8dt)
        lr = lb.reshape(N_DOM, TILES, P)
        oh_u8 = (lr[:, :, :, None] == np.arange(C)[None, None, None, :])
        oh_u8 = (oh_u8.astype(np.uint8) * 0x38).transpose(2, 0, 1, 3)
        ohT_u8 = (lb[:, None, :] == np.arange(C)[None, :, None])
        ohT_u8 = (ohT_u8.astype(np.uint8) * 0x38).transpose(1, 0, 2)
        in_maps.append({
            "feat": np.ascontiguousarray(fc.reshape(P, TT * DS)),
            "oh": np.ascontiguousarray(oh_u8.reshape(P, TT * C)).view(f8dt),
            "ohT": np.ascontiguousarray(ohT_u8.reshape(C, N_DOM * NSH)).view(f8dt),
            "negI": negI,
            "invc": invc_full,
        })
    return in_maps, counts


def finish_host(out_maps, counts, labels=None):
    """Combine per-core outputs into the scalar loss (numpy, float64)."""
    comp = np.zeros(N_DOM, dtype=np.float64)
    for m in out_maps:
        q = m["out_q"].astype(np.float64).reshape(P, N_DOM, TILES)
        dist = np.sqrt(np.maximum(q, 0.0))
        comp += dist.sum(axis=(0, 2))
    comp /= N

    S = out_maps[0]["out_sums"].astype(np.float64)
    cnt = np.maximum(counts, 1.0)
    sep = np.zeros(N_DOM, dtype=np.float64)
    centers = np.zeros((N_DOM, C, D), dtype=np.float64)
    for d in range(N_DOM):
        centers[d] = S[:, d * D:(d + 1) * D] / cnt[d][:, None]
        cd = centers[d]
        sq = ((cd[:, None, :] - cd[None, :, :]) ** 2).sum(-1)
        dist = np.sqrt(np.maximum(sq, 0.0))
        np.fill_diagonal(dist, 0.0)
        sep[d] = dist.sum() / (C * (C - 1))

    intra = (BETA * comp.sum() - ALPHA * sep.sum()) / N_DOM
    inter = 0.0
    n_pairs = 0
    for i in range(N_DOM):
        for j in range(i + 1, N_DOM):
            inter += np.sqrt(((centers[i] - centers[j]) ** 2).sum()) / C
            n_pairs += 1
    inter /= n_pairs
    return np.float32(GAMA * intra + inter)


def kernel(features, labels):
    from concourse.bass_utils import run_bass_kernel_spmd

    nc = _get_nc()
    in_maps, counts = shard_inputs(features, labels)
    res = run_bass_kernel_spmd(nc, in_maps, core_ids=list(range(N_CORES)))
    return finish_host(res.results, counts)
